# revision 13
# baseline (speedup 1.0000x reference)
"""GQA attention forward (dense_transformer), 8-core tensor-parallel Bass kernel.

Problem (hardcoded): B=2, S=1024, H=4096, n_kv=8, G=8 (heads/kv), D=64, f32 io.
Sharding: core m owns kv-group m (8 q-heads + 1 kv-head), computes its slice
attnT_m = [512, 2048] of the attention output (feature-on-partition transposed
layout), AllGathers attnT (bf16, split per batch for overlap), then computes
output columns y[:, m*512:(m+1)*512] = attn @ wd[m*512:(m+1)*512, :].T.
Host concatenates the 8 column slices.

All matmuls run in bf16 (rel-err budget 2e-2); softmax skips max-subtraction
(logits bounded ~|6|); row sums come free from a ones-column appended to V.
"""

import sys

import numpy as np

for _p in ("/opt/trn_rl_repo",):
    if _p not in sys.path:
        sys.path.insert(0, _p)

import ml_dtypes

B, S, H = 2, 1024, 4096
NKV, G, D = 8, 8, 64
NC = 8
BS = B * S          # 2048 flattened tokens
EL = G * D          # 512 local attn features per core
HT = H // 128       # 32 h-tiles
ST = BS // 128      # 16 s-tiles
SB = S // 128       # 8 s-tiles per batch
INV = 0.125         # 1/sqrt(D)

_CACHE = {}


def _fix_bir_for_old_walrus(bir_json):
    """Adapt newer-concourse BIR to the container's older walrus:
    - register allocations need num_physical_regs set
    - only one sem-wait per instruction: hoist extras onto EventSemaphore nops
    """
    import json

    bir = json.loads(bir_json)
    for f in bir["functions"]:
        for a in f["allocations"]:
            if a.get("Skind") == "register" and not a.get("num_physical_regs"):
                a["num_physical_regs"] = 1
        for blk in f["blocks"]:
            newins = []
            for ins in blk["instructions"]:
                si = ins.get("sync_info") or {}
                waits = si.get("on_wait") or []
                if len(waits) > 1:
                    for j, w in enumerate(waits[:-1]):
                        newins.append(
                            {
                                "engine": ins["engine"],
                                "ins": [],
                                "outs": [],
                                "name": f"{ins['name']}_w{j}",
                                "opcode": "EventSemaphore",
                                "sync_info": {"on_update": [], "on_wait": [w]},
                                "debug": ins.get("debug"),
                            }
                        )
                    si["on_wait"] = [waits[-1]]
                newins.append(ins)
            blk["instructions"] = newins
    return json.dumps(bir).encode()


def _install_compiler_shim():
    if _CACHE.get("shim"):
        return
    import concourse.bass_utils as bu
    import concourse.bass2jax as b2j

    orig = bu.compile_bir_kernel.__wrapped__ if hasattr(
        bu.compile_bir_kernel, "__wrapped__"
    ) else bu.compile_bir_kernel

    def patched(bir_json, tmpdir, neff_name="file.neff"):
        return orig(_fix_bir_for_old_walrus(bir_json), tmpdir, neff_name)

    bu.compile_bir_kernel = patched
    b2j.compile_bir_kernel = patched
    _CACHE["shim"] = True


def build():
    _install_compiler_shim()
    import concourse.bass as bass  # noqa: F401
    import concourse.mybir as mybir
    import concourse.tile as tile
    from concourse import bacc

    fp32 = mybir.dt.float32
    bf16 = mybir.dt.bfloat16
    AF = mybir.ActivationFunctionType
    ALU = mybir.AluOpType

    nc = bacc.Bacc("TRN2", debug=False, target_bir_lowering=False, num_devices=NC)

    hsT = nc.declare_dram_parameter("hsT", [H, BS], bf16, isOutput=False)
    wqT = nc.declare_dram_parameter("wqT", [H, EL], bf16, isOutput=False)
    wkT = nc.declare_dram_parameter("wkT", [H, D], bf16, isOutput=False)
    wvT = nc.declare_dram_parameter("wvT", [H, D], bf16, isOutput=False)
    wdT = nc.declare_dram_parameter("wdT", [H, EL], bf16, isOutput=False)
    cosq = nc.declare_dram_parameter("cosq", [128, BS], fp32, isOutput=False)
    sinq = nc.declare_dram_parameter("sinq", [128, BS], fp32, isOutput=False)
    trimask = nc.declare_dram_parameter("trimask", [128, 128], bf16, isOutput=False)
    out = nc.declare_dram_parameter("out", [BS, EL], fp32, isOutput=True)

    rg = [list(range(NC))]

    with tile.TileContext(nc, num_cores=NC) as tc:
        with (
            tc.tile_pool(name="const", bufs=1) as cp,
            tc.tile_pool(name="dram", bufs=1, space="DRAM") as dp,
        ):
            # ---- resident weights / tables ----
            wq_sb = cp.tile([128, HT, EL], bf16)
            wk_sb = cp.tile([128, HT, D], bf16)
            wv_sb = cp.tile([128, HT, D], bf16)
            wd_sb = cp.tile([128, HT, EL], bf16)
            nc.sync.dma_start(wq_sb[:], wqT.ap().rearrange("(a p) e -> p a e", p=128))
            nc.sync.dma_start(wk_sb[:], wkT.ap().rearrange("(a p) e -> p a e", p=128))
            nc.sync.dma_start(wv_sb[:], wvT.ap().rearrange("(a p) e -> p a e", p=128))
            nc.sync.dma_start(wd_sb[:], wdT.ap().rearrange("(a p) e -> p a e", p=128))

            ones_sb = cp.tile([1, 64], bf16)
            nc.gpsimd.memset(ones_sb[:], 1.0)
            tri_sb = cp.tile([128, 128], bf16)
            nc.sync.dma_start(tri_sb[:], trimask.ap())

            # v with a ones column (row sums ride along the PV matmul)
            v_ext = cp.tile([128, ST, D + 1], bf16)
            nc.gpsimd.memset(v_ext[:, :, D : D + 1], 1.0)

            qT_sb = cp.tile([128, 4, BS], bf16)   # q^T, e=g*64+d on partitions
            kT_sb = cp.tile([128, BS], bf16)      # k^T duplicated in both halves
            attnT_b = [cp.tile([128, 4, S], bf16, name=f"attnT{b}") for b in range(B)]

            # AllGather buffers, one per batch so AG(b0) overlaps attention(b1)
            agin = [dp.tile([EL, S], bf16, name=f"agin{b}") for b in range(B)]
            agout = [
                dp.tile([NC * EL, S], bf16, addr_space="Shared", name=f"agout{b}")
                for b in range(B)
            ]

            # ================= Phase 1: QKV projections =================
            with (
                tc.tile_pool(name="proj", bufs=1) as pp,
                tc.tile_pool(name="projpsum", bufs=2, space="PSUM") as ppp,
            ):
                cos_sb = pp.tile([128, BS], fp32)
                sin_sb = pp.tile([128, BS], fp32)
                nc.sync.dma_start(cos_sb[:], cosq.ap())
                nc.sync.dma_start(sin_sb[:], sinq.ap())

                qTraw = pp.tile([128, 4, BS], bf16)
                kTraw = pp.tile([64, BS], bf16)

                hsT_r = hsT.ap().rearrange("(a p) s -> p a s", p=128)
                for st in range(ST):
                    hst = pp.tile([128, HT, 128], bf16, tag="hst", bufs=2)
                    nc.sync.dma_start(hst[:], hsT_r[:, :, st * 128 : (st + 1) * 128])

                    qp = ppp.tile([128, 4, 128], fp32, tag="qp")
                    kp = ppp.tile([64, 128], fp32, tag="kp")
                    vp = ppp.tile([128, D], fp32, tag="vp")
                    for et in range(4):
                        for a in range(HT):
                            nc.tensor.matmul(
                                qp[:, et, :],
                                lhsT=wq_sb[:, a, et * 128 : (et + 1) * 128],
                                rhs=hst[:, a, :],
                                start=(a == 0),
                                stop=(a == HT - 1),
                            )
                    for a in range(HT):
                        nc.tensor.matmul(
                            kp[:, :], lhsT=wk_sb[:, a, :], rhs=hst[:, a, :],
                            start=(a == 0), stop=(a == HT - 1),
                        )
                    for a in range(HT):
                        nc.tensor.matmul(
                            vp[:, :], lhsT=hst[:, a, :], rhs=wv_sb[:, a, :],
                            start=(a == 0), stop=(a == HT - 1),
                        )
                    ssl = slice(st * 128, (st + 1) * 128)
                    for et in range(4):
                        nc.scalar.copy(qTraw[:, et, ssl], qp[:, et, :])
                    nc.scalar.copy(kTraw[:, ssl], kp[:, :])
                    nc.scalar.copy(v_ext[:, st, 0:D], vp[:, :])

                # ---- RoPE:  q' = q*cos + shift32(q)*sin_signed ----
                # (sinq is host-prepared with the rotate-half signs folded in)
                for et in range(4):
                    qsh = pp.tile([128, BS], bf16, tag="qsh", bufs=2)
                    for hh in range(2):
                        for half in range(2):
                            dst = slice(hh * 64 + half * 32, hh * 64 + half * 32 + 32)
                            src = slice(
                                hh * 64 + (1 - half) * 32,
                                hh * 64 + (1 - half) * 32 + 32,
                            )
                            nc.sync.dma_start(qsh[dst, :], qTraw[src, et, :])
                    t1 = pp.tile([128, BS], bf16, tag="t1", bufs=2)
                    t2 = pp.tile([128, BS], bf16, tag="t2", bufs=2)
                    nc.vector.tensor_mul(t1[:], qTraw[:, et, :], cos_sb[:])
                    nc.vector.tensor_mul(t2[:], qsh[:], sin_sb[:])
                    nc.vector.tensor_add(qT_sb[:, et, :], t1[:], t2[:])

                ksh = pp.tile([64, BS], bf16)
                for half in range(2):
                    dst = slice(half * 32, half * 32 + 32)
                    src = slice((1 - half) * 32, (1 - half) * 32 + 32)
                    nc.sync.dma_start(ksh[dst, :], kTraw[src, :])
                kt1 = pp.tile([64, BS], bf16)
                kt2 = pp.tile([64, BS], bf16)
                nc.vector.tensor_mul(kt1[:], kTraw[:], cos_sb[0:64, :])
                nc.vector.tensor_mul(kt2[:], ksh[:], sin_sb[0:64, :])
                nc.vector.tensor_add(kT_sb[0:64, :], kt1[:], kt2[:])
                # duplicate into upper partition half (matmul base-partition rule)
                nc.sync.dma_start(kT_sb[64:128, :], kT_sb[0:64, :])

            # ================= Phase 2: attention =================
            with (
                tc.tile_pool(name="attn", bufs=1) as ap_,
                tc.tile_pool(name="stpsum", bufs=4, space="PSUM") as pst,
                tc.tile_pool(name="pvpsum", bufs=2, space="PSUM") as ppv,
                tc.tile_pool(name="bcpsum", bufs=2, space="PSUM") as pbc,
            ):
                for b in range(B):
                    for g in range(G):
                        qrows = slice((g % 2) * 64, (g % 2) * 64 + 64)
                        for si in range(SB):
                            pv = ppv.tile([D + 1, 128], fp32, tag="pv")
                            scol = slice(b * S + si * 128, b * S + (si + 1) * 128)
                            for ti in range(si + 1):
                                tcol = slice(b * S + ti * 128, b * S + (ti + 1) * 128)
                                stp = pst.tile([128, 128], fp32, tag="st")
                                nc.tensor.matmul(
                                    stp[:],
                                    lhsT=kT_sb[qrows, tcol],
                                    rhs=qT_sb[qrows, g // 2, scol],
                                    start=True,
                                    stop=True,
                                )
                                pT = ap_.tile([128, 128], bf16, tag="pt", bufs=6)
                                nc.scalar.activation(pT[:], stp[:], AF.Exp, scale=INV)
                                if ti == si:
                                    # zero probs where t > s
                                    nc.vector.tensor_mul(pT[:], pT[:], tri_sb[:])
                                nc.tensor.matmul(
                                    pv[:],
                                    lhsT=v_ext[:, b * SB + ti, :],
                                    rhs=pT[:],
                                    start=(ti == 0),
                                    stop=(ti == si),
                                )
                            rc = ap_.tile([1, 128], bf16, tag="rc", bufs=2)
                            with nc.allow_low_precision(reason="softmax recip in bf16"):
                                nc.vector.reciprocal(rc[:], pv[D : D + 1, :])
                            bc = pbc.tile([64, 128], fp32, tag="bc")
                            nc.tensor.matmul(
                                bc[:], lhsT=ones_sb[:], rhs=rc[:], start=True, stop=True
                            )
                            bcs = ap_.tile([64, 128], bf16, tag="bcs", bufs=3)
                            nc.scalar.copy(bcs[:], bc[:])
                            nc.vector.tensor_mul(
                                attnT_b[b][qrows, g // 2, si * 128 : (si + 1) * 128],
                                pv[0:D, :],
                                bcs[:],
                            )
                    # ship this batch's attnT and start its AllGather
                    nc.sync.dma_start(
                        agin[b].rearrange("(a p) s -> p a s", p=128), attnT_b[b][:]
                    )
                    nc.gpsimd.collective_compute(
                        "AllGather",
                        ALU.bypass,
                        replica_groups=rg,
                        ins=[agin[b][:].opt()],
                        outs=[agout[b][:].opt()],
                    )

            # ================= Phase 3: dense (output columns) =================
            with (
                tc.tile_pool(name="dense", bufs=1) as dep,
                tc.tile_pool(name="ypsum", bufs=2, space="PSUM") as pyp,
            ):
                for st in range(ST):
                    b, sl = st // SB, st % SB
                    agr = agout[b].rearrange("(a p) s -> p a s", p=128)
                    agc = dep.tile([128, HT, 128], bf16, tag="agc", bufs=3)
                    nc.sync.dma_start(agc[:], agr[:, :, sl * 128 : (sl + 1) * 128])
                    yp = pyp.tile([128, EL], fp32, tag="yp")
                    for a in range(HT):
                        nc.tensor.matmul(
                            yp[:], lhsT=agc[:, a, :], rhs=wd_sb[:, a, :],
                            start=(a == 0), stop=(a == HT - 1),
                        )
                    ysb = dep.tile([128, EL], fp32, tag="ysb", bufs=2)
                    nc.scalar.copy(ysb[:], yp[:])
                    nc.sync.dma_start(out.ap()[st * 128 : (st + 1) * 128, :], ysb[:])

    nc.finalize()
    return nc


def _prep_inputs(hidden_states, cos, sin, wq, wk, wv, wd):
    bf = ml_dtypes.bfloat16
    hs2 = np.ascontiguousarray(
        hidden_states.reshape(BS, H).T.astype(bf)
    )  # [H, BS]
    cosT = cos.T.astype(np.float32)  # [64, 1024]
    sinT = sin.T.astype(np.float32)
    sinS = np.concatenate([-sinT[0:32], sinT[32:64]], axis=0)
    cosq = np.ascontiguousarray(np.tile(cosT, (2, 2)))  # [128, 2048]
    sinq = np.ascontiguousarray(np.tile(sinS, (2, 2)))
    # probs^T diag-tile mask: keep t <= s  ->  upper triangular incl diag
    tri = np.triu(np.ones((128, 128), dtype=np.float32)).astype(bf)
    in_maps = []
    for m in range(NC):
        in_maps.append(
            {
                "hsT": hs2,
                "wqT": np.ascontiguousarray(
                    wq[m * EL : (m + 1) * EL, :].T.astype(bf)
                ),
                "wkT": np.ascontiguousarray(wk[m * D : (m + 1) * D, :].T.astype(bf)),
                "wvT": np.ascontiguousarray(wv[m * D : (m + 1) * D, :].T.astype(bf)),
                "wdT": np.ascontiguousarray(
                    wd[m * EL : (m + 1) * EL, :].T.astype(bf)
                ),
                "cosq": cosq,
                "sinq": sinq,
                "trimask": tri,
            }
        )
    return in_maps


def kernel(hidden_states, alibi, attention_mask, cos, sin, wq, wk, wv, wd,
           _trace=False):
    from concourse.bass_utils import run_bass_kernel_spmd

    if "nc" not in _CACHE:
        _CACHE["nc"] = build()
    nc = _CACHE["nc"]
    in_maps = _prep_inputs(hidden_states, cos, sin, wq, wk, wv, wd)
    res = run_bass_kernel_spmd(nc, in_maps, core_ids=list(range(NC)), trace=_trace)
    _CACHE["last_result"] = res
    outs = [res.results[m]["out"].reshape(B, S, EL) for m in range(NC)]
    return np.concatenate(outs, axis=-1).astype(np.float32)
